# revision 75
# baseline (speedup 1.0000x reference)
"""Bass/Trainium2 kernel for nn_Attention_46566035423948.

Multi-head attention (B=4, N=2048, C=1024, H=16) on 8 NeuronCores.
Sharding: core c = (batch b = c//2, head-group g = c%2, 8 heads each).
Each core computes a partial projection output [N, C]; the host sums the
two head-group partials per batch and adds b_proj.

Per-core dataflow (v2 — PV runs "transposed" so its matmul cost halves,
and the whole program is one continuous ACT-saturated stream):
  phase 1: Q^T,K^T [128, 2048] bf16 per head-pair (64+64 dims packed on
           partitions), V [2048, 8*65] bf16 with a ones column per head,
           from bf16 xT and W_qkv slices (SCALE pre-folded into Wq).
  phase 2: per (q-block 512, head-pair, k-chunk 128):
           S^T = K^T.T @ Q^T (pair of bf16 matmuls into PE quadrants)
           E = exp(S^T) (ScalarE, PSUM->SBUF bf16)
           P = E * maskT (one DVE broadcast-AP multiply for both heads)
           PV transposed: P^T[k,q-128] is the PE *stationary*; the moving
           operand is V-aug [k, 65] -> accumulates x[q 128, 64+denom] in
           PSUM (rows = 65 per chunk instead of 512). One PSUM bank per
           head holds 4 q-tile accumulators (zero-region start trick +
           skip_group_check).
           Normalize: accumulators staged to SBUF (frees the banks), DVE
           reciprocal of the denom column + tensor_scalar multiply ->
           xn [q 128, 128] bf16 (two heads side by side); PE-transpose
           (identity) -> xnT [d 128, q] for the projection.
  phase 3: out = xnT.T @ W_proj_slice (bf16), staged through SBUF (DVE)
           to DRAM.

Scheduling: the S->exp->mask chain runs back-to-back across all 16
(pair, q-block) units so the ScalarE exp stream (the 266 us engine
floor) never pauses; the PV stream trails it by SKEW chunks via a FIFO.
All other PE work (QK production for upcoming units, proj groups,
normalize/transpose steps) drips in a few steps per k-chunk from an
injection queue. The warmup interleaves the first two units' S-chains
so ACT overlaps the PE-heavy V/K/Q production.
"""

import numpy as np
import ml_dtypes

import concourse.mybir as mybir
import concourse.tile as tile
from concourse import bacc
from concourse import bass_utils

N_CORES = 8
B, N, C, H = 4, 2048, 1024, 16
HS = C // H           # 64
SCALE = HS ** -0.5
HPC = 8               # heads per core
GW = HPC * HS         # 512: per-core head-group width
PAIRS = 4             # head pairs per core
CC = C // 128         # 8 contraction chunks over C
KC = N // 128         # 16 key chunks
QB = N // 512         # 4 query blocks of 512
QC = N // 128         # 16 query chunks of 128 (proj)
SKEW = 8              # PV trails S by SKEW k-chunks

F32 = mybir.dt.float32
BF16 = mybir.dt.bfloat16
EXP = mybir.ActivationFunctionType.Exp

_NC_CACHE = []


MASK_AHEAD = 4  # mask tiles DMA'd this many k-chunks ahead of use


def _emit(tc, xT, wq, wk, wv, mT, wp, ident_d, out):
    nc = tc.nc
    from contextlib import ExitStack

    with ExitStack() as stack:
        # persistent pools: V lives through phase 2, xnT through phase 3
        v_pool = stack.enter_context(tc.tile_pool(name="vp", bufs=KC))
        xnt_pool = stack.enter_context(tc.tile_pool(name="xnt", bufs=1))
        wp_pool = stack.enter_context(tc.tile_pool(name="wpp", bufs=PAIRS))
        ostage_pool = stack.enter_context(tc.tile_pool(name="ostage", bufs=6))
        id_pool = stack.enter_context(tc.tile_pool(name="idp", bufs=1))

        v_t = []
        xnT = [xnt_pool.tile([128, N], BF16, name=f"xnT{i}", tag=f"xnT{i}")
               for i in range(PAIRS)]
        wp_t = []
        mask_ctr = [0]

        with tc.tile_pool(name="qkt", bufs=4) as qkt_pool, \
             tc.tile_pool(name="ep", bufs=6) as e_pool, \
             tc.tile_pool(name="pp", bufs=20) as p_pool, \
             tc.tile_pool(name="rinv", bufs=4) as rinv_pool, \
             tc.tile_pool(name="xnq", bufs=8) as xn_pool, \
             tc.tile_pool(name="xst", bufs=4) as xst_pool, \
             tc.tile_pool(name="xt", bufs=1) as xt_pool, \
             tc.tile_pool(name="wqk", bufs=4) as wqk_pool, \
             tc.tile_pool(name="wvp", bufs=1) as wv_pool, \
             tc.tile_pool(name="mp", bufs=12) as m_pool, \
             tc.tile_pool(name="ps2s", bufs=2, space="PSUM") as s_pool, \
             tc.tile_pool(name="ps1", bufs=2, space="PSUM") as ps1_pool, \
             tc.tile_pool(name="psacc", bufs=2, space="PSUM") as acc_pool:

            ident = id_pool.tile([128, 128], BF16, name="ident")
            nc.sync.dma_start(ident[:], ident_d)

            def dma_w(which, wsrc, pair, wts):
                wt = wqk_pool.tile([128, CC * 128], BF16, name="wqk_t",
                                   tag="wqk_t")
                nc.sync.dma_start(wt[:], wsrc[pair])
                for cc in range(CC):
                    wts[(which, cc)] = wt[:, cc * 128:(cc + 1) * 128]

            def dma_wqk(pair):
                wts = {}
                dma_w("k", wk, pair, wts)
                dma_w("q", wq, pair, wts)
                return wts

            # DMA in PE-consumption order: pair-0 K weights, the 8 qb=0
            # xt chunks and wv first.
            wts0 = {}
            wt0 = wqk_pool.tile([128, CC * 128], BF16, name="wqk_t",
                                tag="wqk_t")
            nc.sync.dma_start(wt0[:, 0:CC * 64], wk[0][:, 0:CC * 64])
            nc.sync.dma_start(wt0[:, CC * 64:], wk[0][:, CC * 64:])
            for cc in range(CC):
                wts0[("k", cc)] = wt0[:, cc * 128:(cc + 1) * 128]
            xt_q = {}
            wv_t = []
            t = xt_pool.tile([128, CC * 512], BF16, name="xt_0")
            half = CC * 256
            quart = CC * 128
            for j in range(4):
                nc.sync.dma_start(t[:, j * quart:(j + 1) * quart],
                                  xT[0, :, j * quart:(j + 1) * quart])
            for cc in range(CC):
                xt_q[(cc, 0)] = t[:, cc * 512:(cc + 1) * 512]
            dma_w("q", wq, 0, wts0)
            t = wv_pool.tile([128, CC * 512], BF16, name="wv_all")
            nc.sync.dma_start(t[:, 0:half], wv[:, 0:half])
            nc.sync.dma_start(t[:, half:], wv[:, half:])
            for cc in range(CC):
                wv_t.append(t[:, cc * 512:(cc + 1) * 512])
            for qb in range(1, QB):
                t = xt_pool.tile([128, CC * 512], BF16, name=f"xt_{qb}")
                nc.sync.dma_start(t[:], xT[qb])
                for cc in range(CC):
                    xt_q[(cc, qb)] = t[:, cc * 512:(cc + 1) * 512]

            # V tiles are emitted lazily inside the first attention block's
            # k-loop. Layout [128, 8*65]: head h at cols h*65..h*65+64 plus
            # a ones column at h*65+64 (accumulates softmax row sums).
            def emit_v(kc):
                ps = ps1_pool.tile([128, 512], F32, name="ps1t", tag="ps1t")
                for cc in range(CC):
                    nc.tensor.matmul(
                        ps[:],
                        xt_q[(cc, kc // 4)][:, (kc % 4) * 128:
                                            (kc % 4) * 128 + 128],
                        wv_t[cc],
                        start=(cc == 0), stop=(cc == CC - 1))
                t = v_pool.tile([128, HPC * (HS + 1)], BF16, name="v_t",
                                tag="v_t")
                tv = t[:].rearrange("p (h d) -> p h d", h=HPC)
                nc.gpsimd.memset(tv[:, :, HS:HS + 1], 1.0)
                nc.vector.tensor_copy(
                    tv[:, :, 0:HS],
                    ps[:].rearrange("p (h d) -> p h d", h=HPC))
                v_t.append(t)

            proj_done = []

            def proj_group(qc, nh):
                ps = ps1_pool.tile([128, 512], F32, name="ps1t", tag="ps1t")
                for pair_ in range(PAIRS):
                    nc.tensor.matmul(
                        ps[:], xnT[pair_][:, qc * 128:(qc + 1) * 128],
                        wp_t[pair_][:, nh * 512:(nh + 1) * 512],
                        start=(pair_ == 0), stop=(pair_ == PAIRS - 1))
                ost = ostage_pool.tile([128, 512], F32, name="ost", tag="ost")
                nc.vector.tensor_copy(ost[:], ps[:])
                nc.sync.dma_start(
                    out[qc * 128:(qc + 1) * 128, nh * 512:(nh + 1) * 512],
                    ost[:])
                proj_done.append((qc, nh))

            # Injection queue: all auxiliary PE work (QK production for
            # upcoming units, proj groups, deferred normalize/transpose
            # steps) is dripped into the attention k-loops a few steps per
            # k-chunk so PE never bursts while ACT idles. Steps that open a
            # ps1-tag PSUM accumulation group are pushed as contiguous
            # bundles; FIFO order keeps the 2-slot ring interleave-free
            # (which would otherwise deadlock the in-order engine queues).
            from collections import deque
            inj = deque()
            wts_by_pair = {0: wts0}
            qkt = {}

            def alloc_qkt(pair_):
                for which in ("q", "k"):
                    qkt[(which, pair_)] = qkt_pool.tile(
                        [128, N], BF16, name="qkt_t", tag="qkt_t")

            def qk_group(which, pair_, qb_):
                ps = ps1_pool.tile([128, 512], F32, name="ps1t",
                                   tag="ps1t")
                for cc in range(CC):
                    nc.tensor.matmul(
                        ps[:], wts_by_pair[pair_][(which, cc)][:],
                        xt_q[(cc, qb_)][:],
                        start=(cc == 0), stop=(cc == CC - 1))
                nc.vector.tensor_copy(
                    qkt[(which, pair_)][:, qb_ * 512:(qb_ + 1) * 512],
                    ps[:])

            def push_qk(which, pair_, qb_):
                cell = {}

                def mk(cc):
                    def f():
                        if cc == 0:
                            cell["ps"] = ps1_pool.tile(
                                [128, 512], F32, name="ps1t", tag="ps1t")
                        nc.tensor.matmul(
                            cell["ps"][:],
                            wts_by_pair[pair_][(which, cc)][:],
                            xt_q[(cc, qb_)][:],
                            start=(cc == 0), stop=(cc == CC - 1))
                    return f

                def cp():
                    nc.vector.tensor_copy(
                        qkt[(which, pair_)][:, qb_ * 512:(qb_ + 1) * 512],
                        cell["ps"][:])
                for cc in range(CC):
                    inj.append(mk(cc))
                inj.append(cp)

            def push_proj(qc, nh):
                cell = {}

                def mk(p0):
                    def f():
                        if p0 == 0:
                            cell["ps"] = ps1_pool.tile(
                                [128, 512], F32, name="ps1t", tag="ps1t")
                        for pair_ in (p0, p0 + 1):
                            nc.tensor.matmul(
                                cell["ps"][:],
                                xnT[pair_][:, qc * 128:(qc + 1) * 128],
                                wp_t[pair_][:, nh * 512:(nh + 1) * 512],
                                start=(pair_ == 0),
                                stop=(pair_ == PAIRS - 1))
                    return f

                def fin():
                    ost = ostage_pool.tile([128, 512], F32, name="ost",
                                           tag="ost")
                    nc.vector.tensor_copy(ost[:], cell["ps"][:])
                    nc.sync.dma_start(
                        out[qc * 128:(qc + 1) * 128,
                            nh * 512:(nh + 1) * 512], ost[:])
                inj.extend([mk(0), mk(2), fin])
                proj_done.append((qc, nh))

            # ---- unified stream: the S->exp->mask chain runs continuously
            # across all 16 (pair, q-block) units; the PV stream trails it
            # by SKEW chunks via a FIFO. Unit ends never pause the S stream
            # (ACT stays saturated). After a unit's last PV, its two PSUM
            # accumulator banks are immediately staged to SBUF (freeing
            # them for the next unit) and normalize/transpose steps join
            # the drip queue.
            units = [(p_, q_) for p_ in range(PAIRS) for q_ in range(QB)]
            pv_q = {u_: deque() for u_ in range(len(units))}
            pv_total = [0]
            pop_u = [0]
            finalized = set()
            acc_of = {}
            mt_of = {}

            def load_mask_tile(u, kc):
                pair, qb = units[u]
                mt = m_pool.tile([128, 512], BF16, name="m_t", tag="m_t")
                nc.sync.dma_start(mt[:], mT[qb, kc])
                mt_of.setdefault(u, {})[kc] = mt

            def s_chain(u, kc, mtiles):
                pair, qb = units[u]
                qs = slice(qb * 512, (qb + 1) * 512)
                KT = qkt[("k", pair)]
                QT = qkt[("q", pair)]
                ks = slice(kc * 128, (kc + 1) * 128)
                S = s_pool.tile([128, 1024], F32, name="S", tag="S")
                nc.tensor.matmul(S[:, 0:512], KT[0:64, ks], QT[0:64, qs],
                                 start=True, stop=True, tile_position=(0, 0))
                nc.tensor.matmul(S[:, 512:1024], KT[64:128, ks],
                                 QT[64:128, qs],
                                 start=True, stop=True, tile_position=(64, 0))
                E = e_pool.tile([128, 1024], BF16, name="E", tag="E")
                nc.scalar.activation(E[:], S[:], EXP)
                P = p_pool.tile([128, 1024], BF16, name="P", tag="P")
                # one multiply for both heads: mask broadcast over head dim
                mt = mtiles.pop(kc)
                mb_ = mt[:].unsqueeze(1).broadcast_to([128, 2, 512])
                e2 = E[:].rearrange("p (h q) -> p h q", h=2)
                p2 = P[:].rearrange("p (h q) -> p h q", h=2)
                nc.vector.tensor_mul(p2, e2, mb_)
                mask_ctr[0] += 1
                pv_q[u].append((kc, P))
                pv_total[0] += 1

            def finalize(u):
                """Stage the accumulators to SBUF (frees the PSUM banks),
                then queue normalize + transpose steps."""
                pair, qb = units[u]
                acc = acc_of.pop(u)
                xst = [xst_pool.tile([128, 4 * 65], F32, name="xst",
                                     tag="xst") for _ in range(2)]
                for h in range(2):
                    nc.vector.tensor_copy(
                        xst[h][:].rearrange("p (qt c) -> p qt c", qt=4),
                        acc[h][:].rearrange("p (qt c) -> p qt c",
                                            qt=4)[:, :, 0:65])
                xn_q = [xn_pool.tile([128, 128], BF16, name="xn_q",
                                     tag="xn_q") for _ in range(4)]

                def norm_step(h, qt):
                    def f():
                        r = rinv_pool.tile([128, 1], F32, name="r", tag="r")
                        nc.vector.reciprocal(
                            r[:], xst[h][:, qt * 65 + 64:qt * 65 + 65])
                        nc.vector.tensor_scalar_mul(
                            xn_q[qt][:, h * 64:(h + 1) * 64],
                            xst[h][:, qt * 65:qt * 65 + 64], r[:])
                    return f

                def tpose_step(qt):
                    def f():
                        tp = ps1_pool.tile([128, 128], BF16, name="tp",
                                           tag="ps1t")
                        nc.tensor.transpose(tp[:], xn_q[qt][:], ident[:])
                        nc.vector.tensor_copy(
                            xnT[pair][:, qb * 512 + qt * 128:
                                       qb * 512 + (qt + 1) * 128],
                            tp[:])
                    return f

                for h in range(2):
                    for qt in range(4):
                        inj.append(norm_step(h, qt))
                for qt in range(4):
                    inj.append(tpose_step(qt))

            def try_pop():
                """PV one chunk of the oldest unfinished unit, if its PSUM
                accumulators are (or can become) live."""
                u = pop_u[0]
                if u >= len(units) or not pv_q[u]:
                    return False
                if u not in acc_of:
                    if u > 0 and (u - 1) not in finalized:
                        return False
                    acc_of[u] = [acc_pool.tile([128, 512], F32, name="acc",
                                               tag="acc") for _ in range(2)]
                kc, P = pv_q[u].popleft()
                pv_total[0] -= 1
                pair, qb = units[u]
                acc = acc_of[u]
                for h in range(2):
                    h65 = (2 * pair + h) * 65
                    for qt in range(4):
                        nc.tensor.matmul(
                            acc[h][:, qt * 128:qt * 128 + 65],
                            P[:, h * 512 + qt * 128:h * 512 + (qt + 1) * 128],
                            v_t[kc][:, h65:h65 + 65],
                            start=(kc == 0 and qt == 0), stop=(kc == KC - 1),
                            tile_position=(0, 0), skip_group_check=True)
                if kc == KC - 1:
                    finalize(u)
                    finalized.add(u)
                    pop_u[0] += 1
                return True

            # pair-0 bootstrap: K/Q(0) inline; later K-groups, Q(1) and V
            # production run inside the first unit's k-loop
            warm_drips = [0]
            alloc_qkt(0)
            qk_group("k", 0, 0)
            qk_group("q", 0, 0)

            def pre0(kc):
                if kc in (1, 5, 9):
                    qk_group("k", 0, kc // 4 + 1)
                if kc == 2:
                    qk_group("q", 0, 1)
                emit_v(kc)

            def unit_start(u):
                pair, qb = units[u]
                if pair == 1 and qb == 0:
                    # prefetch proj weights once SBUF headroom exists
                    for pp_ in range(PAIRS):
                        t = wp_pool.tile([128, C], BF16, name="wp_t",
                                         tag="wp_t")
                        nc.sync.dma_start(t[:], wp[pp_])
                        wp_t.append(t)
                if qb + 1 < QB and u != 0:
                    push_qk("q", pair, qb + 1)
                if pair + 1 < PAIRS:
                    if qb == 0:
                        wts_by_pair[pair + 1] = dma_wqk(pair + 1)
                    if qb == 1:
                        alloc_qkt(pair + 1)
                        push_qk("k", pair + 1, 0)
                        push_qk("k", pair + 1, 1)
                    if qb == 2:
                        push_qk("k", pair + 1, 2)
                        push_qk("k", pair + 1, 3)
                    if qb == 3:
                        push_qk("q", pair + 1, 0)
                for j in range(MASK_AHEAD):
                    load_mask_tile(u, j)

            # warmup: units 0 and 1 (same pair) interleave their S-chains
            # so ACT has twice the exp work to overlap the PE-heavy V/K/Q
            # production; PV pops favor the oldest unit and catch up at
            # 2/step afterwards.
            sched = [(0, 0), (0, 1), (0, 2)]
            w0 = [(0, kc) for kc in range(3, KC)]
            w1 = [(1, kc) for kc in range(KC)]
            for i in range(KC):
                sched.append(w1[i])
                if i < len(w0):
                    sched.append(w0[i])
            for u_ in range(2, len(units)):
                sched.extend((u_, kc) for kc in range(KC))

            for u, kc in sched:
                pair, qb = units[u]
                if kc == 0:
                    unit_start(u)
                if u == 0:
                    pre0(kc)
                if pair == PAIRS - 1 and qb >= 1 and kc == 10:
                    # by now the previous unit's transpose steps are in
                    # the FIFO, so its q-block's proj can queue behind
                    done = set(proj_done)
                    for c, n_ in [(c, n) for c in range(qb * 4)
                                  for n in range(2)
                                  if (c, n) not in done]:
                        push_proj(c, n_)
                if kc + MASK_AHEAD < KC:
                    load_mask_tile(u, kc + MASK_AHEAD)
                s_chain(u, kc, mt_of[u])
                npop = 3 if pv_total[0] > SKEW + 8 else \
                    (2 if pv_total[0] > SKEW + 2 else
                     (1 if pv_total[0] > SKEW else 0))
                for _ in range(npop):
                    if not try_pop():
                        break
                # during warmup drip only Q(0,2)'s 9 steps (needed at unit
                # 2); the pair-1 K-groups (needed at unit 4) defer to the
                # spare capacity of units 2-3
                if u == 0:
                    rate = 0
                elif u == 1:
                    rate = 1 if warm_drips[0] < 9 else 0
                    warm_drips[0] += rate
                else:
                    rate = 3
                for _ in range(rate):
                    if inj:
                        inj.popleft()()

            while pv_total[0]:
                if not try_pop():
                    raise RuntimeError("pv drain stuck")
                for _ in range(2):
                    if inj:
                        inj.popleft()()
            while inj:
                inj.popleft()()

            # remaining proj groups (qb3's q-chunks + any not streamed)
            done = set(proj_done)
            for qc in range(QC):
                for nh in range(2):
                    if (qc, nh) not in done:
                        proj_group(qc, nh)


def build():
    if _NC_CACHE:
        return _NC_CACHE[0]
    nc = bacc.Bacc("TRN2", target_bir_lowering=False, debug=False,
                   enable_asserts=False, num_devices=N_CORES)
    xT = nc.dram_tensor("xT", [QB, 128, CC * 512], BF16,
                        kind="ExternalInput").ap()
    wq = nc.dram_tensor("wq", [PAIRS, 128, CC * 128], BF16,
                        kind="ExternalInput").ap()
    wk = nc.dram_tensor("wk", [PAIRS, 128, CC * 128], BF16,
                        kind="ExternalInput").ap()
    wv = nc.dram_tensor("wv", [128, CC * 512], BF16,
                        kind="ExternalInput").ap()
    mT = nc.dram_tensor("mT", [QB, KC, 128, 512], BF16,
                        kind="ExternalInput").ap()
    wp = nc.dram_tensor("wp", [PAIRS, 128, C], BF16,
                        kind="ExternalInput").ap()
    ident_d = nc.dram_tensor("ident", [128, 128], BF16,
                             kind="ExternalInput").ap()
    out = nc.dram_tensor("out", [N, C], F32, kind="ExternalOutput").ap()
    with tile.TileContext(nc) as tc:
        _emit(tc, xT, wq, wk, wv, mT, wp, ident_d, out)
    nc.compile()
    _NC_CACHE.append(nc)
    return nc


def _tile4(a, rows, cols):
    """[R, Q] -> [Q//cols, R//rows, rows, cols] contiguous tiles so every
    device DMA is a single contiguous transfer."""
    R, Q = a.shape
    return np.ascontiguousarray(
        a.reshape(R // rows, rows, Q // cols, cols).transpose(0, 2, 1, 3)
         .transpose(1, 0, 2, 3))


def _pack_cc(a, cols):
    """[C, Q] -> [Q//cols, 128, (C//128)*cols]: per q-block, the 8
    contraction chunks side by side on 128 partitions (one contiguous DMA
    per q-block)."""
    R, Q = a.shape
    t = a.reshape(R // 128, 128, Q // cols, cols)      # [cc, p, qb, c]
    return np.ascontiguousarray(
        t.transpose(2, 1, 0, 3).reshape(Q // cols, 128, (R // 128) * cols))


def shard_inputs(joint_feature, mask, W_qkv, W_proj, b_proj):
    mT = _tile4(np.ascontiguousarray(mask[0, 0].T).astype(ml_dtypes.bfloat16),
                128, 512)
    ident = np.eye(128, dtype=ml_dtypes.bfloat16)
    in_maps = []
    for c in range(N_CORES):
        b, g = divmod(c, 2)
        lo, hi = g * GW, (g + 1) * GW
        in_maps.append({
            "xT": _pack_cc(np.ascontiguousarray(joint_feature[b].T)
                           .astype(ml_dtypes.bfloat16), 512),
            "wq": _pack_cc((W_qkv[:, lo:hi] * SCALE)
                           .astype(ml_dtypes.bfloat16), 128),
            "wk": _pack_cc(W_qkv[:, C + lo:C + hi]
                           .astype(ml_dtypes.bfloat16), 128),
            "wv": _pack_cc(W_qkv[:, 2 * C + lo:2 * C + hi]
                           .astype(ml_dtypes.bfloat16), 512)[0],
            "mT": mT,
            "wp": np.ascontiguousarray(
                W_proj[lo:hi, :].reshape(PAIRS, 128, C))
                .astype(ml_dtypes.bfloat16),
            "ident": ident,
        })
    return in_maps


def kernel(joint_feature, mask, W_qkv, W_proj, b_proj):
    joint_feature = np.asarray(joint_feature, dtype=np.float32)
    mask = np.asarray(mask)
    W_qkv = np.asarray(W_qkv, dtype=np.float32)
    W_proj = np.asarray(W_proj, dtype=np.float32)
    b_proj = np.asarray(b_proj, dtype=np.float32)

    nc = build()
    in_maps = shard_inputs(joint_feature, mask, W_qkv, W_proj, b_proj)
    res = bass_utils.run_bass_kernel_spmd(nc, in_maps,
                                          core_ids=list(range(N_CORES)))
    out = np.empty((B, N, C), dtype=np.float32)
    for b in range(B):
        out[b] = res.results[2 * b]["out"] + res.results[2 * b + 1]["out"] \
            + b_proj
    return out
